# revision 11
# baseline (speedup 1.0000x reference)
"""CorefHead Trainium2 kernel.

Reference computation (B=64, S=512, H=1024, HID=512):
  emb_a = span_mean(bert, offsets[:,0:2])   # [B,H]
  emb_b = span_mean(bert, offsets[:,2:4])   # [B,H]
  emb_p = bert[b, offsets[:,4]]             # [B,H]
  x = concat([emb_a, emb_b, emb_p], -1)     # [B,3H]
  h = leaky_relu(batchnorm_eval(x @ W1 + b1), 0.01)
  out = h @ W2 + b2                         # [B,3]

Strategy: pure data parallel, batch sharded 8 ways (8 batches/core).
Host packs, per core, only the rows covered by the two spans (union,
deduped, densely packed; rows of spans shorter than 32 ship bf16, the
rest fp8 e3m4 -- a short span's embedding is an O(1) vector, so 2% fp8
noise on it would blow the 2e-2 budget, while long spans average it
away).

mm1 keeps the span-membership masks [128, nch, 16] (8 batches x 2
spans) STATIONARY on the PE (16-column weight loads are ~free; bert
chunks stream as the moving operand), accumulating xT' [16, 256] per
h-quarter in PSUM.  bert ships as 4 separate h-quarter streams so each
quarter's PSUM evacuation (DVE copy -> PE transpose -> 1/len scale ->
mm2 span matmuls) pipelines under the next quarter's DMA.  pron rows
ship pre-transposed bf16 and DMA straight into the xT tile; mm2
accumulates h[8, 512] from xT chunks against W1 (bf16, BN scale folded
on host; pron chunks first = early PE work).  Head: per 128-wide hid
chunk: PE transpose, +BN bias, LeakyReLU (DVE), mm3 accumulate [3, 8]
(PE), +b2, DMA out.  Host gathers per-core [3, 8] outputs and undoes
the batch permutation.
"""

import numpy as np

B, S, H = 64, 512, 1024
HID = 512
EPS = 1e-5
NCORES = 8
BPC = B // NCORES   # batches per core
HC = H // 128       # 8 h-chunks per embedding
NQ = 4              # h-quarter streams
QW = H // NQ        # 256 cols per quarter
NMC = 2 * BPC       # mask columns: 2 spans x 8 batches
SHORT = 32          # spans shorter than this ship bf16 rows

TRACE = False
LAST_RESULT = None

_PROGRAM_CACHE: dict = {}


def _build_program(nf: int, nb: int):
    """Build + compile the SPMD Bass program: nf fp8 row-chunks and nb
    bf16 row-chunks per core."""
    import concourse.bacc as bacc
    import concourse.tile as tile
    import concourse.mybir as mybir
    from concourse.bass import MemorySpace
    from concourse.masks import make_identity

    f32 = mybir.dt.float32
    bf16 = mybir.dt.bfloat16
    f8 = mybir.dt.float8e3

    nc = bacc.Bacc("TRN2", target_bir_lowering=False, debug=False,
                   num_devices=NCORES)

    bFq_d = [nc.dram_tensor(f"bF{q}", [128, nf, QW], f8,
                            kind="ExternalInput").ap() for q in range(NQ)]
    maskF_d = nc.dram_tensor("maskF", [128, nf, NMC], f8, kind="ExternalInput").ap()
    if nb:
        bertB_d = nc.dram_tensor("bertB", [128, nb, H], bf16, kind="ExternalInput").ap()
        maskB_d = nc.dram_tensor("maskB", [128, nb, NMC], bf16, kind="ExternalInput").ap()
    pronT_d = nc.dram_tensor("pronT", [128, HC, BPC], bf16, kind="ExternalInput").ap()
    sfac_d = nc.dram_tensor("sfac", [128, 2, BPC], f32, kind="ExternalInput").ap()
    # hconsts cols: 0:4 bnb (BN bias, [128] per hid chunk), 4:16 w2
    # ([128, 3] per hid chunk), 16 b2 (rows 0:3)
    hco_d = nc.dram_tensor("hconsts", [128, 17], f32, kind="ExternalInput").ap()
    # W1 (BN scale folded) [128, kc, HID]; kc 0..7 pron (e=2), 8..23 spans
    w1_d = nc.dram_tensor("w1P", [128, 3 * HC, HID], bf16, kind="ExternalInput").ap()
    out_d = nc.dram_tensor("out", [3, BPC], f32, kind="ExternalOutput").ap()

    with tile.TileContext(nc) as tc:
        with (
            tc.tile_pool(name="singles", bufs=1) as singles,
            tc.tile_pool(name="psum_px", bufs=2, space=MemorySpace.PSUM) as psum_px,
            tc.tile_pool(name="psum_h", bufs=1, space=MemorySpace.PSUM) as psum_h,
            tc.tile_pool(name="psum_t", bufs=2, space=MemorySpace.PSUM) as psum_t,
            tc.tile_pool(name="psum_ht", bufs=2, space=MemorySpace.PSUM) as psum_ht,
        ):
            # --- ACT ring: consts, pron, maskB, W1 in 2 pieces ---
            sfac_t = singles.tile([128, 2, BPC], f32)
            nc.scalar.dma_start(out=sfac_t, in_=sfac_d)
            hco_t = singles.tile([128, 17], f32)
            nc.scalar.dma_start(out=hco_t, in_=hco_d)
            xT_t = singles.tile([128, 3, HC, BPC], bf16)
            nc.scalar.dma_start(out=xT_t[:, 2, :, :], in_=pronT_d)
            if nb:
                maskB_t = singles.tile([128, nb, NMC], bf16)
                nc.scalar.dma_start(out=maskB_t, in_=maskB_d)
                bertB_t = singles.tile([128, nb, H], bf16)
                nc.scalar.dma_start(out=bertB_t, in_=bertB_d)
            w1_t = singles.tile([128, 3 * HC, HID], bf16)
            nc.scalar.dma_start(out=w1_t[:, 8:24, :], in_=w1_d[:, 8:24, :])

            # --- SP ring: pron W1, masks, then the four h-quarter streams
            # (bert lands last overall; span mm2s are xT-gated, and the
            # post-DMA tail is only quarter 3's chain + head) ---
            nc.sync.dma_start(out=w1_t[:, 0:8, :], in_=w1_d[:, 0:8, :])
            maskF_t = singles.tile([128, nf, NMC], f8)
            nc.sync.dma_start(out=maskF_t, in_=maskF_d)
            bFq_t = []
            for q in range(NQ):
                bq = singles.tile([128, nf, QW], f8, tag=f"bF{q}")
                nc.sync.dma_start(out=bq, in_=bFq_d[q])
                bFq_t.append(bq)

            idt = singles.tile([16, 16], f32)
            make_identity(nc, idt)

            # x' staging: pxs[e*8+b, h] (x transposed; h on free dim)
            pxs_t = singles.tile([16, H], f32)

            ph = psum_h.tile([BPC, HID], f32)

            # PE warm-up: ~4us of junk matmuls (only dependent on the
            # identity tile) so the HAM clock-gate releases to 2.4 GHz
            # before the real matmul stream begins.  They scribble on ph,
            # which the first real mm2 (start=True) zeroes anyway.
            for _ in range(48):
                nc.tensor.matmul(ph[:, 0:16], idt[:, 0:BPC], idt,
                                 start=True, stop=True)

            nmm2 = 3 * HC
            mm2i = 0

            def mm2(e, hc):
                nonlocal mm2i
                kc = ((e + 1) % 3) * HC + hc  # w1P kc order: e=2,0,1
                nc.tensor.matmul(
                    ph, xT_t[:, e, hc, :], w1_t[:, kc, :],
                    start=(mm2i == 0), stop=(mm2i == nmm2 - 1))
                mm2i += 1

            for q in range(NQ):
                # mm1 for this h-quarter: masks stationary, bert moving
                px = psum_px.tile([16, QW], f32, tag="px")
                nmm = nb + nf
                k = 0
                for sc in range(nb):
                    nc.tensor.matmul(
                        px, maskB_t[:, sc, :],
                        bertB_t[:, sc, q * QW:(q + 1) * QW],
                        start=(k == 0), stop=(k == nmm - 1))
                    k += 1
                for sc in range(nf):
                    nc.tensor.matmul(
                        px, maskF_t[:, sc, :], bFq_t[q][:, sc, :],
                        start=(k == 0), stop=(k == nmm - 1))
                    k += 1

                if q == 0:
                    # pron mm2 chunks: early PE work while quarter 1 streams
                    for hc in range(HC):
                        mm2(2, hc)

                # evacuate: copy, transpose, scale into xT, span mm2
                for hh in range(2):
                    hc = 2 * q + hh
                    nc.vector.tensor_copy(
                        pxs_t[:, hc * 128:(hc + 1) * 128],
                        px[:, hh * 128:(hh + 1) * 128])
                    pt = psum_t.tile([128, 2, BPC], f32, tag="pt")
                    nc.tensor.transpose(
                        pt, pxs_t[:, hc * 128:(hc + 1) * 128], idt)
                    nc.vector.tensor_mul(xT_t[:, 0:2, hc, :], pt, sfac_t)
                    mm2(0, hc)
                    mm2(1, hc)

            # --- head: transpose h chunks, BN bias, LeakyReLU, mm3 ---
            ot_ps = psum_h.tile([3, BPC], f32, tag="oT")
            for mc in range(HID // 128):
                hs_t = singles.tile([BPC, 128], f32, tag="hs")
                nc.vector.tensor_copy(hs_t, ph[:, mc * 128:(mc + 1) * 128])
                pht = psum_ht.tile([128, BPC], f32, tag="pht")
                nc.tensor.transpose(pht, hs_t, idt[0:BPC, 0:BPC])
                t_t = singles.tile([128, BPC], f32, tag="t_t")
                nc.vector.tensor_scalar_add(t_t, pht, hco_t[:, mc:mc + 1])
                y_t = singles.tile([128, BPC], f32, tag="y_t")
                nc.vector.scalar_tensor_tensor(
                    y_t, t_t, 0.01, t_t,
                    op0=mybir.AluOpType.mult, op1=mybir.AluOpType.max)
                nc.tensor.matmul(
                    ot_ps, hco_t[:, 4 + 3 * mc:7 + 3 * mc], y_t,
                    start=(mc == 0), stop=(mc == HID // 128 - 1))

            o_t = singles.tile([3, BPC], f32)
            nc.vector.tensor_scalar_add(o_t, ot_ps, hco_t[0:3, 16:17])
            nc.sync.dma_start(out=out_d, in_=o_t)

    nc.compile()
    return nc


def _pack_rows(offsets):
    """Per batch: (fp8_rows, bits_f, bf16_rows, bits_b, (la, lb)); bits
    uint8 with bit0 = span A membership, bit1 = span B."""
    out = []
    for gb in range(offsets.shape[0]):
        a0, a1, b0, b1_, p = (int(v) for v in offsets[gb])
        la, lb = a1 - a0 + 1, b1_ - b0 + 1
        pos = np.arange(S)
        in_a = (pos >= a0) & (pos <= a1)
        in_b = (pos >= b0) & (pos <= b1_)
        used = in_a | in_b
        short = ((in_a & (la < SHORT)) | (in_b & (lb < SHORT)))
        rows_b = np.nonzero(used & short)[0]
        rows_f = np.nonzero(used & ~short)[0]
        bits = in_a.astype(np.uint8) | (in_b.astype(np.uint8) << 1)
        out.append((rows_f, bits[rows_f], rows_b, bits[rows_b], (la, lb)))
    return out


def kernel(bert_outputs, offsets, W1, b1, gamma, beta, running_mean,
           running_var, W2, b2):
    import ml_dtypes

    f8np = ml_dtypes.float8_e3m4
    bfnp = ml_dtypes.bfloat16

    bert = np.ascontiguousarray(np.asarray(bert_outputs, dtype=np.float32))
    offs = np.asarray(offsets).astype(np.int64)
    W1 = np.asarray(W1, dtype=np.float32)
    b1 = np.asarray(b1, dtype=np.float32)
    gamma = np.asarray(gamma, dtype=np.float32)
    beta = np.asarray(beta, dtype=np.float32)
    rm = np.asarray(running_mean, dtype=np.float32)
    rv = np.asarray(running_var, dtype=np.float32)
    W2 = np.asarray(W2, dtype=np.float32)
    b2 = np.asarray(b2, dtype=np.float32)

    # Fold BN eval-mode stats: bn(xW1 + b1) = x(W1*s) + ((b1 - mean)*s + beta)
    s = gamma / np.sqrt(rv + EPS)
    biasv = (b1 - rm) * s + beta
    W1s = (W1 * s[None, :]).astype(bfnp)
    w1sp = W1s.reshape(3, HC, 128, HID)
    w1P = np.ascontiguousarray(
        np.concatenate([w1sp[2], w1sp[0], w1sp[1]], axis=0).transpose(1, 0, 2))
    hco = np.zeros((128, 17), dtype=np.float32)
    hco[:, 0:4] = biasv.reshape(HID // 128, 128).T
    hco[:, 4:16] = W2.reshape(HID // 128, 128, 3).transpose(1, 0, 2).reshape(128, 12)
    hco[0:3, 16] = b2

    packs = _pack_rows(offs)

    # Greedy balance batches over cores by row load (8 per core).
    nfr = np.array([len(p[0]) for p in packs])
    nbr = np.array([len(p[2]) for p in packs])
    order = np.argsort(-(nfr + nbr), kind="stable")
    loads = [0] * NCORES
    counts = [0] * NCORES
    perm = np.empty((BPC, NCORES), dtype=np.int64)
    for gb in order:
        c = min((i for i in range(NCORES) if counts[i] < BPC),
                key=lambda i: loads[i])
        perm[counts[c], c] = gb
        loads[c] += nfr[gb] + nbr[gb]
        counts[c] += 1

    nf = max(1, -(-max(int(sum(nfr[perm[:, c]])) for c in range(NCORES)) // 128))
    nb = -(-max(int(sum(nbr[perm[:, c]])) for c in range(NCORES)) // 128)

    key = (nf, nb)
    if key not in _PROGRAM_CACHE:
        _PROGRAM_CACHE[key] = _build_program(nf, nb)
    nc = _PROGRAM_CACHE[key]

    in_maps = []
    for c in range(NCORES):
        bertF = np.zeros((nf * 128, H), dtype=np.float32)
        maskF = np.zeros((nf * 128, NMC), dtype=f8np)
        bertB = np.zeros((max(nb, 1) * 128, H), dtype=bfnp)
        maskB = np.zeros((max(nb, 1) * 128, NMC), dtype=bfnp)
        pronT = np.empty((BPC, H), dtype=np.float32)
        sfac = np.ones((2, BPC), dtype=np.float32)
        rf = rb = 0
        for slot in range(BPC):
            gb = perm[slot, c]
            rows_f, bits_f, rows_b, bits_b, (la, lb) = packs[gb]
            m = len(rows_f)
            bertF[rf:rf + m] = bert[gb, rows_f]
            maskF[rf:rf + m, slot] = (bits_f & 1)
            maskF[rf:rf + m, BPC + slot] = (bits_f >> 1)
            rf += m
            m = len(rows_b)
            bertB[rb:rb + m] = bert[gb, rows_b]
            maskB[rb:rb + m, slot] = (bits_b & 1)
            maskB[rb:rb + m, BPC + slot] = (bits_b >> 1)
            rb += m
            pronT[slot] = bert[gb, offs[gb, 4]]
            sfac[0, slot] = 1.0 / la
            sfac[1, slot] = 1.0 / lb
        bertF8 = bertF.astype(f8np).reshape(nf, 128, H).transpose(1, 0, 2)
        in_map = {
            "maskF": np.ascontiguousarray(
                maskF.reshape(nf, 128, NMC).transpose(1, 0, 2)),
            "pronT": np.ascontiguousarray(
                pronT.T.reshape(HC, 128, BPC).transpose(1, 0, 2).astype(bfnp)),
            "sfac": np.broadcast_to(sfac, (128, 2, BPC)).copy(),
            "hconsts": hco,
            "w1P": w1P,
        }
        for q in range(NQ):
            in_map[f"bF{q}"] = np.ascontiguousarray(
                bertF8[:, :, q * QW:(q + 1) * QW])
        if nb:
            in_map["bertB"] = np.ascontiguousarray(
                bertB.reshape(nb, 128, H).transpose(1, 0, 2))
            in_map["maskB"] = np.ascontiguousarray(
                maskB.reshape(nb, 128, NMC).transpose(1, 0, 2))
        in_maps.append(in_map)

    from concourse import bass_utils
    kwargs = {}
    if TRACE:
        kwargs = {"trace": True, "trace_cores": list(range(NCORES))}
    res = bass_utils.run_bass_kernel_spmd(nc, in_maps,
                                          core_ids=list(range(NCORES)),
                                          **kwargs)
    global LAST_RESULT
    LAST_RESULT = res

    out = np.empty((B, 3), dtype=np.float32)
    for c in range(NCORES):
        out[perm[:, c]] = res.results[c]["out"].T
    return out
